# revision 2
# baseline (speedup 1.0000x reference)
"""Trainium2 Bass kernel for batched 2-D Gaussian KDE.

reference:
    pdf[b, i] = norm * sum_j exp(-||c_i - c_j||^2 / (2 sigma^2)) * w[b, j]
    with B=8, N=4096, coordinates [B, N, 2], norm = 1/(2 pi sigma^2).

Strategy
--------
Data-parallel over B: one batch element per NeuronCore (8 cores).

Per core, flash-style over j-blocks: the N x N pairwise matrix is never
materialized in DRAM.  The exp argument is produced by a single TensorE
matmul per tile:

    M[i, j] = x_i x_j + y_i y_j + 1 * v_j,   v_j = -|c_j|^2/2 + sigma^2 ln w_j

so that  exp((1/sigma^2) M + bias_i) = norm * w_j * exp(-d2/(2 sigma^2))
with bias_i = -|c_i|^2/(2 sigma^2) + ln norm.

FP32 matmuls run at 1/4 rate on the PE, so each fp32 coordinate is split
exactly into 3 bf16 terms (8-bit mantissa each; 3 terms cover the full 24-bit
fp32 mantissa).  Keeping the 6 product terms >= 2^-27 gives a K=15 bf16
contraction that runs at full PE rate with abs error ~3e-8 on M (1.2e-5 on
the exp argument after the 1/sigma^2 scale).

ScalarE evaluates exp in-place on PSUM and its accum_out port emits the
row-sum per 2048-wide tile, so pdf falls out of the activation directly:
no separate reduction pass over the N x N tile is needed.
"""

import sys

sys.path.insert(0, "/opt/trn_rl_repo")

import numpy as np
import ml_dtypes

B = 8
N = 4096
NB = N // 128  # 32 i-blocks of 128
JG = 2048  # j-group width handled by one activation (4 PSUM banks)
NJG = N // JG  # 2
KROWS = 15

_COMPILED = None
_LAST_RESULT = None


def _bf16(a):
    return a.astype(ml_dtypes.bfloat16)


def _split3(a64):
    """Exact-ish 3-term bf16 decomposition of a float array (f64 in)."""
    h = _bf16(a64)
    l = _bf16(a64 - h.astype(np.float64))
    ll = _bf16(a64 - h.astype(np.float64) - l.astype(np.float64))
    return h, l, ll


def _build(rep=1):
    import contextlib

    import concourse.tile as tile
    from concourse import bacc, mybir

    f32 = mybir.dt.float32
    bf16 = mybir.dt.bfloat16

    nc = bacc.Bacc("TRN2", target_bir_lowering=False, debug=False, num_devices=B)

    L_d = nc.dram_tensor("L", [KROWS, N], bf16, kind="ExternalInput").ap()
    R_d = nc.dram_tensor("R", [KROWS, N], bf16, kind="ExternalInput").ap()
    bias_d = nc.dram_tensor("bias", [128, NB], f32, kind="ExternalInput").ap()
    scale_d = nc.dram_tensor("scale", [128, 1], f32, kind="ExternalInput").ap()
    out_d = nc.dram_tensor("out", [128, NB], f32, kind="ExternalOutput").ap()

    with tile.TileContext(nc) as tc:
        with (
            tc.tile_pool(name="sbuf", bufs=1) as pool,
            tc.tile_pool(name="psum", bufs=2, space="PSUM") as psum,
        ):
            L_sb = pool.tile([KROWS, N], bf16)
            R_sb = pool.tile([KROWS, N], bf16)
            bias_sb = pool.tile([128, NB], f32)
            scale_sb = pool.tile([128, 1], f32)
            parts = pool.tile([128, NB * NJG], f32)
            final = pool.tile([128, NB], f32)

            nc.sync.dma_start(L_sb[:], L_d[:])
            nc.sync.dma_start(R_sb[:], R_d[:])
            nc.sync.dma_start(bias_sb[:], bias_d[:])
            nc.sync.dma_start(scale_sb[:], scale_d[:])

            loop = tc.For_i(0, rep, 1) if rep > 1 else contextlib.nullcontext()
            with loop:
                for ib in range(NB):
                    lhs = L_sb[:, ib * 128 : (ib + 1) * 128]
                    for g in range(NJG):
                        ps = psum.tile([128, JG], f32)
                        for s in range(JG // 512):
                            j0 = g * JG + s * 512
                            nc.tensor.matmul(
                                ps[:, s * 512 : (s + 1) * 512],
                                lhs,
                                R_sb[:, j0 : j0 + 512],
                                start=True,
                                stop=True,
                            )
                        col = ib * NJG + g
                        nc.scalar.activation(
                            ps[:],
                            ps[:],
                            mybir.ActivationFunctionType.Exp,
                            bias=bias_sb[:, ib : ib + 1],
                            scale=scale_sb[:, 0:1],
                            accum_out=parts[:, col : col + 1],
                        )

                nc.vector.reduce_sum(
                    final[:],
                    parts[:].rearrange("p (a b) -> p a b", b=NJG),
                    axis=mybir.AxisListType.X,
                )
                nc.sync.dma_start(out_d[:], final[:])

    nc.compile()
    return nc


def _prep_core(xy, w, sigma):
    """Host-side prep for one batch element -> input map for one core."""
    x = xy[:, 0].astype(np.float64)
    y = xy[:, 1].astype(np.float64)
    w64 = np.maximum(w.astype(np.float64), 1e-35)
    sig2 = float(sigma) ** 2
    c = 1.0 / (2.0 * sig2)
    lognorm = -np.log(2.0 * np.pi * sig2)
    sq = x * x + y * y
    v = -0.5 * sq + sig2 * np.log(w64)

    xh, xl, xll = _split3(x)
    yh, yl, yll = _split3(y)
    vh, vl, vll = _split3(v)
    one = np.ones(N, dtype=ml_dtypes.bfloat16)

    # pairs (i-side, j-side): (h,h) (h,l) (l,h) (h,ll) (ll,h) (l,l) per coord
    Lrows = [xh, xh, xl, xh, xll, xl, yh, yh, yl, yh, yll, yl, one, one, one]
    Rrows = [xh, xl, xh, xll, xh, xl, yh, yl, yh, yll, yh, yl, vh, vl, vll]
    L = np.stack(Lrows).astype(ml_dtypes.bfloat16)
    R = np.stack(Rrows).astype(ml_dtypes.bfloat16)

    bias = (-c * sq + lognorm).astype(np.float32).reshape(NB, 128).T.copy()
    scale = np.full((128, 1), 1.0 / sig2, dtype=np.float32)
    return {"L": L, "R": R, "bias": bias, "scale": scale}


def kernel(weights, coordinates, sigma):
    global _COMPILED, _LAST_RESULT
    from concourse.bass_utils import run_bass_kernel_spmd

    if _COMPILED is None:
        _COMPILED = _build()
    nc = _COMPILED

    in_maps = [
        _prep_core(np.asarray(coordinates[b]), np.asarray(weights[b]), sigma)
        for b in range(B)
    ]
    res = run_bass_kernel_spmd(nc, in_maps, list(range(B)))
    _LAST_RESULT = res

    pdf = np.empty((B, N), dtype=np.float32)
    for b in range(B):
        out = res.results[b]["out"]  # [128, 32]
        pdf[b] = out.T.reshape(N)
    return pdf
